# revision 23
# baseline (speedup 1.0000x reference)
"""Causal self-attention (b=2, t=2048, d=1024, h=16) on 8 trn2 NeuronCores.

Sharding: core c handles batch c//4 and the 4 heads 4*(c%4)..4*(c%4)+3
(data parallel over batch x tensor parallel over heads). Each core
computes x @ w_qkv for its head-slice, causal attention for its heads,
and a partial out-projection y_heads @ w_out[head_rows]; the host sums
the 4 partial outputs per batch (the tensor-parallel all-reduce).

v2: bf16 end-to-end (rel err ~6e-3, gate is 2e-2).
  - all inputs loaded via DMA-transpose (2-byte dtype) split across the
    sync + scalar HWDGE queues; no hi/lo split, no staging copies.
  - causal mask folded into the S PSUM accumulation as a matmul with a
    constant -BIG upper-triangle (maskT @ I), so exp(scale*(S+mask))=0
    above the diagonal -- no DVE masking pass.
  - one fused exp per j-chunk over both heads ([128, 2, 512] PSUM tile).
  - softmax denominator via fused ones-column in V (row 64 of the PV
    accumulator); renorm = ACT reciprocal + gpsimd partition_broadcast
    + DVE multiply.
  - ib-outer loop with the out-projection interleaved one half-block
    behind attention, so the PE stream stays dense (HAM warm).
"""

import numpy as np
import ml_dtypes

import concourse.bacc as bacc
import concourse.mybir as mybir
import concourse.tile as tile
from concourse.bass_utils import run_bass_kernel_spmd

F32 = mybir.dt.float32
BF16 = mybir.dt.bfloat16

T = 2048            # sequence length
D = 1024            # model dim
DH = 64             # head dim
HPC = 4             # heads per core
NCORES = 8
NTT = T // 128      # 16 t-tiles of 128
NDC = D // 128      # 8 d-chunks of 128
NIB = T // 512      # 4 i-blocks of 512
JPB = 512 // 128    # j-chunks per i-block
BIG = 30000.0


def _build():
    nc = bacc.Bacc("TRN2", target_bir_lowering=False, debug=False)

    XT = nc.dram_tensor("XT", [D, T], BF16, kind="ExternalInput")
    WQKV = nc.dram_tensor("WQKV", [D, 768], BF16, kind="ExternalInput")
    WO = nc.dram_tensor("WO", [256, D], BF16, kind="ExternalInput")
    MASKT = nc.dram_tensor("MASKT", [128, 128], BF16, kind="ExternalInput")
    IDENT = nc.dram_tensor("IDENT", [128, 128], BF16, kind="ExternalInput")
    OUT = nc.dram_tensor("OUT", [T, D], BF16, kind="ExternalOutput")

    EXP = mybir.ActivationFunctionType.Exp
    RECIP = mybir.ActivationFunctionType.Reciprocal

    with tile.TileContext(nc) as tc:
        with tc.tile_pool(name="persist", bufs=1) as pp, \
             tc.tile_pool(name="pt", bufs=4) as ppt, \
             tc.tile_pool(name="prec", bufs=2) as prec, \
             tc.tile_pool(name="pyc", bufs=4) as pyc, \
             tc.tile_pool(name="pbc", bufs=2) as pbc, \
             tc.tile_pool(name="post", bufs=3) as post, \
             tc.tile_pool(name="psS", bufs=2, space="PSUM") as psS, \
             tc.tile_pool(name="psY", bufs=2, space="PSUM") as psY, \
             tc.tile_pool(name="psO", bufs=2, space="PSUM") as psO:

            xt = pp.tile([128, NDC, T], BF16, tag="xt")
            wsb = pp.tile([128, NDC, 768], BF16, tag="wsb")
            wo_sb = pp.tile([128, 2, D], BF16, tag="wo")
            qt = [pp.tile([128, T], BF16, tag=f"qt{p}", name=f"qt{p}")
                  for p in range(2)]
            kt = [pp.tile([128, T], BF16, tag=f"kt{p}", name=f"kt{p}")
                  for p in range(2)]
            vones = pp.tile([128, NTT, HPC, DH + 1], BF16, tag="vones")
            ypair = [pp.tile([128, T], BF16, tag=f"yp{p}", name=f"yp{p}")
                     for p in range(2)]
            maskt = pp.tile([128, 128], BF16, tag="maskt")
            ident = pp.tile([128, 128], BF16, tag="ident")

            # ---- input DMAs ----
            # x is pre-transposed on the host, so every load is a plain
            # contiguous DMA; spread across the two HWDGE queues.
            nc.sync.dma_start(maskt[:], MASKT[:])
            nc.sync.dma_start(ident[:], IDENT[:])
            nc.gpsimd.memset(vones[:, :, :, DH:DH + 1], 1.0)
            for dc in range(NDC):
                nc.gpsimd.dma_start(
                    wsb[:, dc, :],
                    WQKV[dc * 128:(dc + 1) * 128, :])
                q = nc.sync if dc % 2 == 0 else nc.scalar
                q.dma_start(
                    xt[:, dc, :], XT[dc * 128:(dc + 1) * 128, :])
            for pi in range(2):
                nc.gpsimd.dma_start(
                    wo_sb[:, pi, :], WO[pi * 128:(pi + 1) * 128, :])

            # PE warm-up: dense dummy matmuls while the input DMAs stream,
            # so HAM un-throttles before the real work arrives.
            warm = psO.tile([128, 512], F32, tag="op", name="warm")
            for _ in range(56):
                nc.tensor.matmul(
                    warm[:, 0:128], maskt[:], ident[:],
                    start=True, stop=True)

            # ---- phase A helpers ----
            def emit_qk(pi):
                for base, dst in ((0, qt[pi]), (256, kt[pi])):
                    for ib in range(NIB):
                        qp = psS.tile([128, 512], F32, tag="stab")
                        for dc in range(NDC):
                            nc.tensor.matmul(
                                qp[:],
                                wsb[:, dc, base + pi * 128:base + (pi + 1) * 128],
                                xt[:, dc, ib * 512:(ib + 1) * 512],
                                start=(dc == 0), stop=(dc == NDC - 1))
                        nc.vector.tensor_copy(
                            dst[:, ib * 512:(ib + 1) * 512], qp[:])

            def emit_v(t0, t1):
                for ti in range(t0, t1):
                    vp = psS.tile([128, 256], F32, tag="stab")
                    for dc in range(NDC):
                        nc.tensor.matmul(
                            vp[:], xt[:, dc, ti * 128:(ti + 1) * 128],
                            wsb[:, dc, 512:768],
                            start=(dc == 0), stop=(dc == NDC - 1))
                    nc.vector.tensor_copy(
                        vones[:, ti, :, 0:DH],
                        vp[:].rearrange("p (h d) -> p h d", h=HPC))

            # ---- phase B block: attention for (ib, pi) ----
            def emit_attn(ib, pi):
                ya = psY.tile([65, 512], F32, tag="y", name="ya")
                yb = psY.tile([65, 512], F32, tag="y", name="yb")
                njc = JPB * ib + JPB

                def emit_pv(ptab, jc, off):
                    nc.tensor.matmul(
                        ya[0:65, off:512], vones[:, jc, 2 * pi, :],
                        ptab[:, 0, off:512],
                        start=(jc == 0), stop=(jc == njc - 1),
                        skip_group_check=True)
                    nc.tensor.matmul(
                        yb[0:65, off:512], vones[:, jc, 2 * pi + 1, :],
                        ptab[:, 1, off:512],
                        start=(jc == 0), stop=(jc == njc - 1),
                        skip_group_check=True)
                pending = None
                for jc in range(njc):
                    diag = jc >= JPB * ib
                    off = 128 * (jc - JPB * ib) if diag else 0
                    stab = psS.tile([128, 2, 512], F32, tag="stab")
                    ptab = ppt.tile([128, 2, 512], BF16, tag="ptab")
                    js = slice(jc * 128, (jc + 1) * 128)
                    isl = slice(ib * 512 + off, (ib + 1) * 512)
                    nc.tensor.matmul(
                        stab[:, 0, off:512], kt[pi][0:64, js],
                        qt[pi][0:64, isl], start=True, stop=not diag,
                        skip_group_check=True)
                    nc.tensor.matmul(
                        stab[:, 1, off:512], kt[pi][64:128, js],
                        qt[pi][64:128, isl], start=True, stop=not diag,
                        tile_position=(64, 0), skip_group_check=True)
                    if diag:
                        nc.tensor.matmul(
                            stab[:, 0, off:off + 128], maskt[:], ident[:],
                            start=False, stop=True, skip_group_check=True)
                        nc.tensor.matmul(
                            stab[:, 1, off:off + 128], maskt[:], ident[:],
                            start=False, stop=True, skip_group_check=True)
                    nc.scalar.activation(
                        ptab[:, :, off:512], stab[:, :, off:512],
                        EXP, scale=0.125)
                    if pending is not None:
                        emit_pv(*pending)
                    pending = (ptab, jc, off)
                emit_pv(*pending)
                # renorm: y /= denom (row 64), both heads at once
                # drain the PV accumulators to SBUF right away so the
                # PSUM banks free for the next block; the renorm chain then
                # runs entirely off the SBUF copy.
                yca = pyc.tile([65, 512], F32, tag="yc", name="yca")
                ycb = pyc.tile([65, 512], F32, tag="yc", name="ycb")
                nc.vector.tensor_copy(yca[:], ya[:])
                nc.vector.tensor_copy(ycb[:], yb[:])
                den_sb = prec.tile([1, 2, 512], F32, tag="den")
                rec = prec.tile([1, 2, 512], F32, tag="rec")
                bcs = pbc.tile([64, 2, 512], F32, tag="bcs")
                nc.vector.tensor_copy(den_sb[:, 0, :], yca[64:65, :])
                nc.vector.tensor_copy(den_sb[:, 1, :], ycb[64:65, :])
                nc.vector.reciprocal_approx_fast(rec[:], den_sb[:])
                nc.gpsimd.partition_broadcast(bcs[:], rec[:])
                ibs = slice(ib * 512, (ib + 1) * 512)
                nc.vector.tensor_mul(
                    ypair[pi][0:64, ibs], yca[0:64, :], bcs[0:64, 0, :])
                nc.vector.tensor_mul(
                    ypair[pi][64:128, ibs], ycb[0:64, :], bcs[0:64, 1, :])

            # ---- phase C block: out-projection for i-block ib ----
            def emit_outproj(ib):
                for ti in range(JPB * ib, JPB * ib + JPB):
                    ost = post.tile([128, D], BF16, tag="ost")
                    for eh in range(2):
                        op = psO.tile([128, 512], F32, tag="op")
                        nc.tensor.matmul(
                            op[:], ypair[0][:, ti * 128:(ti + 1) * 128],
                            wo_sb[:, 0, eh * 512:(eh + 1) * 512],
                            start=True, stop=False)
                        nc.tensor.matmul(
                            op[:], ypair[1][:, ti * 128:(ti + 1) * 128],
                            wo_sb[:, 1, eh * 512:(eh + 1) * 512],
                            start=False, stop=True)
                        nc.vector.tensor_copy(
                            ost[:, eh * 512:(eh + 1) * 512], op[:])
                    nc.sync.dma_start(
                        OUT[ti * 128:(ti + 1) * 128, :], ost[:])

            # ---- emission schedule ----
            # A(pi0) -> B(0,0) -> A(pi1) -> B(0,1) -> v tail per ib;
            # out-proj for ib lands one half-block behind its renorm so
            # the PE queue never waits on the renorm chain.
            emit_qk(0)
            emit_v(0, 4)
            emit_attn(0, 0)
            emit_qk(1)
            emit_attn(0, 1)
            emit_v(4, 8)
            emit_attn(1, 0)
            emit_attn(1, 1)
            with tc.high_priority(offset=-150):
                emit_outproj(0)
            emit_v(8, 12)
            emit_attn(2, 0)
            emit_attn(2, 1)
            with tc.high_priority(offset=-150):
                emit_outproj(1)
            emit_v(12, 16)
            emit_attn(3, 0)
            emit_attn(3, 1)
            with tc.high_priority(offset=-150):
                emit_outproj(2)
            emit_outproj(3)

    nc.compile()
    return nc


_NC = None


def build_in_maps(x, w_qkv, w_out):
    x = np.asarray(x, np.float32)
    w_qkv = np.asarray(w_qkv, np.float32)
    w_out = np.asarray(w_out, np.float32)

    idx = np.arange(128)
    maskt = np.where(idx[None, :] > idx[:, None], -BIG, 0.0).astype(
        ml_dtypes.bfloat16)                       # maskt[i,j] = -BIG iff j>i
    identm = np.eye(128, dtype=ml_dtypes.bfloat16)

    in_maps = []
    for c in range(NCORES):
        b, g = divmod(c, 4)
        cs = slice(g * 256, (g + 1) * 256)
        wq = w_qkv[:, 0 * 1024:1 * 1024][:, cs]
        wk = w_qkv[:, 1 * 1024:2 * 1024][:, cs]
        wv = w_qkv[:, 2 * 1024:3 * 1024][:, cs]
        wqkv = np.ascontiguousarray(
            np.concatenate([wq, wk, wv], axis=1)).astype(ml_dtypes.bfloat16)
        wo = np.ascontiguousarray(
            w_out[g * 256:(g + 1) * 256, :]).astype(ml_dtypes.bfloat16)
        in_maps.append({
            "XT": np.ascontiguousarray(x[b].T).astype(ml_dtypes.bfloat16),
            "WQKV": wqkv,
            "WO": wo,
            "MASKT": maskt,
            "IDENT": identm,
        })
    return in_maps


def kernel(x, w_qkv, w_out):
    global _NC
    if _NC is None:
        _NC = _build()

    in_maps = build_in_maps(x, w_qkv, w_out)
    res = run_bass_kernel_spmd(_NC, in_maps, core_ids=list(range(NCORES)))
    outs = [res.results[c]["OUT"].astype(np.float32) for c in range(NCORES)]
    y = np.stack([outs[0] + outs[1] + outs[2] + outs[3],
                  outs[4] + outs[5] + outs[6] + outs[7]], axis=0)
    return y.astype(np.float32)
